# revision 1
# baseline (speedup 1.0000x reference)
"""Trainium2 Bass kernel for nn_AwareDecoder segment first/last gather.

Problem: input [16, 2048, 1024] f32, number_mask [16, 2048] int64 with ids in
[0, 512]. For each segment id i in [0, 512): find first/last row-major token
position with that id, gather those rows of the flattened input, concat ->
out [512, 2048] f32.

Strategy (8 NeuronCores, segment-sharded - no collectives):
  core c owns segments [64c, 64c+64). Each core:
    - DMAs the (tiny, 256KB) id array, extracts int64 low words,
    - computes per-segment min/max token position with an fp16 eq/select/
      reduce sweep on the vector engine. Token chunks sit on partitions and
      positions are encoded chunk-LOCALLY (values <= 256, fp16-exact) so the
      four mult/reduce passes run in the DVE 2x packed mode; the global
      position is reconstructed in the tiny post-transpose stage,
    - PE-transpose + free-axis reduce for the cross-partition combine,
    - gathers its 64 first + 64 last rows (4KB each) straight from HBM with
      one hardware indirect DMA (reads only 512KB of the 128MB input),
    - writes its [64, 2048] slice of the output.
Host concatenates the 8 slices.
"""
import numpy as np

import concourse.bass as bass
import concourse.tile as tile
from concourse import bacc, mybir
from concourse import bass_utils
from concourse.masks import make_identity

P = 128            # partitions
L = 32768          # B*S tokens
H = 1024           # hidden
NSEG = 512         # segments
NCORES = 8
SEG_PER_CORE = NSEG // NCORES            # 64
TOK_PER_PART = L // P                    # 256 tokens per partition
F32 = mybir.dt.float32
F16 = mybir.dt.float16
I32 = mybir.dt.int32


def build_nc():
    nc = bacc.Bacc("TRN2", target_bir_lowering=False, debug=False)

    x = nc.dram_tensor("x", [L, H], F32, kind="ExternalInput")
    # number_mask int64 raw bytes as int32 (lo, hi) pairs; partition p covers
    # tokens [p*256, (p+1)*256).
    idpairs = nc.dram_tensor("idpairs", [P, TOK_PER_PART, 2], I32, kind="ExternalInput")
    # packed fp16 consts (per-core): [c8hi (8*256) | c8lo (8*256) | posmin | posmax]
    cpack_in = nc.dram_tensor("cpack", [P, 18 * TOK_PER_PART], F16,
                              kind="ExternalInput")
    # global-position bases for the post-transpose decode:
    # rows 0..63   (min side): base[s, p] = (127 - p) * 256
    # rows 64..127 (max side): base[s, p] = p * 256
    base_in = nc.dram_tensor("base", [2, SEG_PER_CORE, P], F32, kind="ExternalInput")
    out = nc.dram_tensor("out", [SEG_PER_CORE, 2 * H], F32, kind="ExternalOutput")

    with tile.TileContext(nc) as tc:
        with tc.tile_pool(name="sb", bufs=1) as sb, \
             tc.tile_pool(name="big", bufs=1) as big, \
             tc.tile_pool(name="ps", bufs=1, space="PSUM") as ps:

            # ---- load ids, extract low int32 words, cast to fp16 ----
            idp_t = sb.tile([P, TOK_PER_PART, 2], I32)
            nc.sync.dma_start(idp_t[:], idpairs.ap())
            cpack = sb.tile([P, 18 * TOK_PER_PART], F16)
            nc.scalar.dma_start(cpack[:], cpack_in.ap())
            c8hi_t = cpack[:, 0:8 * TOK_PER_PART].rearrange(
                "p (a t) -> p a t", a=8)
            c8lo_t = cpack[:, 8 * TOK_PER_PART:16 * TOK_PER_PART].rearrange(
                "p (a t) -> p a t", a=8)
            posmin = cpack[:, 16 * TOK_PER_PART:17 * TOK_PER_PART]
            posmax = cpack[:, 17 * TOK_PER_PART:18 * TOK_PER_PART]
            base_t = sb.tile([P, P], F32)
            nc.gpsimd.dma_start(base_t[:], base_in.ap().rearrange("a s p -> (a s) p"))

            # ---- factorized seg compare: id>>3 == base/8 + m, id&7 == lo ----
            hi_i = sb.tile([P, TOK_PER_PART], I32)
            nc.vector.tensor_scalar(hi_i[:], idp_t[:, :, 0], 3, None,
                                    op0=mybir.AluOpType.arith_shift_right)
            lo_i = sb.tile([P, TOK_PER_PART], I32)
            nc.vector.tensor_scalar(lo_i[:], idp_t[:, :, 0], 7, None,
                                    op0=mybir.AluOpType.bitwise_and)
            hi_f = sb.tile([P, TOK_PER_PART], F16)
            nc.vector.tensor_copy(hi_f[:], hi_i[:])
            lo_f = sb.tile([P, TOK_PER_PART], F16)
            nc.vector.tensor_copy(lo_f[:], lo_i[:])

            eq_hi = sb.tile([P, 8, TOK_PER_PART], F16)
            nc.vector.tensor_tensor(
                out=eq_hi[:],
                in0=hi_f[:].unsqueeze(1).broadcast_to([P, 8, TOK_PER_PART]),
                in1=c8hi_t, op=mybir.AluOpType.is_equal)
            eq_lo = sb.tile([P, 8, TOK_PER_PART], F16)
            nc.vector.tensor_tensor(
                out=eq_lo[:],
                in0=lo_f[:].unsqueeze(1).broadcast_to([P, 8, TOK_PER_PART]),
                in1=c8lo_t, op=mybir.AluOpType.is_equal)
            eqlo_min = sb.tile([P, 8, TOK_PER_PART], F16)
            nc.vector.tensor_tensor(
                out=eqlo_min[:], in0=eq_lo[:],
                in1=posmin.unsqueeze(1).broadcast_to([P, 8, TOK_PER_PART]),
                op=mybir.AluOpType.mult)
            eqlo_max = sb.tile([P, 8, TOK_PER_PART], F16)
            nc.vector.tensor_tensor(
                out=eqlo_max[:], in0=eq_lo[:],
                in1=posmax.unsqueeze(1).broadcast_to([P, 8, TOK_PER_PART]),
                op=mybir.AluOpType.mult)

            # ---- big fused candidate passes (2x) + reduces ----
            cand = big.tile([P, 8, 8, TOK_PER_PART], F16)
            nc.vector.tensor_tensor(
                out=cand[:],
                in0=eq_hi[:].unsqueeze(2).broadcast_to([P, 8, 8, TOK_PER_PART]),
                in1=eqlo_min[:].unsqueeze(1).broadcast_to([P, 8, 8, TOK_PER_PART]),
                op=mybir.AluOpType.mult)
            # TT-max tree (2x) then small reduce: 256 -> 32 -> 1
            red = sb.tile([P, P], F16)  # [:, :64] min-enc, [:, 64:] max-enc
            c3 = cand[:].rearrange("p a b t -> p (a b) t")
            lv1 = big.tile([P, SEG_PER_CORE, 128], F16, tag="lv1")
            nc.vector.tensor_tensor(out=lv1[:], in0=c3[:, :, 0:128],
                                    in1=c3[:, :, 128:256], op=mybir.AluOpType.max)
            lv2 = sb.tile([P, SEG_PER_CORE, 64], F16, tag="lv2")
            nc.vector.tensor_tensor(out=lv2[:], in0=lv1[:, :, 0:64],
                                    in1=lv1[:, :, 64:128], op=mybir.AluOpType.max)
            lv3 = sb.tile([P, SEG_PER_CORE, 32], F16, tag="lv3")
            nc.vector.tensor_tensor(out=lv3[:], in0=lv2[:, :, 0:32],
                                    in1=lv2[:, :, 32:64], op=mybir.AluOpType.max)
            nc.vector.tensor_reduce(red[:, 0:SEG_PER_CORE], lv3[:],
                                    axis=mybir.AxisListType.X,
                                    op=mybir.AluOpType.max)
            cand2 = big.tile([P, 8, 8, TOK_PER_PART], F16)
            nc.vector.tensor_tensor(
                out=cand2[:],
                in0=eq_hi[:].unsqueeze(2).broadcast_to([P, 8, 8, TOK_PER_PART]),
                in1=eqlo_max[:].unsqueeze(1).broadcast_to([P, 8, 8, TOK_PER_PART]),
                op=mybir.AluOpType.mult)
            c3b = cand2[:].rearrange("p a b t -> p (a b) t")
            lv1b = big.tile([P, SEG_PER_CORE, 128], F16, tag="lv1")
            nc.vector.tensor_tensor(out=lv1b[:], in0=c3b[:, :, 0:128],
                                    in1=c3b[:, :, 128:256], op=mybir.AluOpType.max)
            lv2b = sb.tile([P, SEG_PER_CORE, 64], F16, tag="lv2")
            nc.vector.tensor_tensor(out=lv2b[:], in0=lv1b[:, :, 0:64],
                                    in1=lv1b[:, :, 64:128], op=mybir.AluOpType.max)
            lv3b = sb.tile([P, SEG_PER_CORE, 32], F16, tag="lv3")
            nc.vector.tensor_tensor(out=lv3b[:], in0=lv2b[:, :, 0:32],
                                    in1=lv2b[:, :, 32:64], op=mybir.AluOpType.max)
            nc.vector.tensor_reduce(red[:, SEG_PER_CORE:P], lv3b[:],
                                    axis=mybir.AxisListType.X,
                                    op=mybir.AluOpType.max)

            # ---- cross-partition combine, decode, gather ----
            ident = sb.tile([P, P], F16)
            make_identity(nc, ident[:])
            red_t = ps.tile([P, P], F16)
            nc.tensor.transpose(out=red_t[:], in_=red[:], identity=ident[:])
            mask = sb.tile([P, P], F32)
            nc.vector.tensor_scalar(mask[:], red_t[:], 0.0, None,
                                    op0=mybir.AluOpType.is_gt)
            glob = sb.tile([P, P], F32)
            nc.vector.tensor_tensor(out=glob[:], in0=red_t[:], in1=base_t[:],
                                    op=mybir.AluOpType.add)
            nc.vector.tensor_tensor(out=glob[:], in0=glob[:], in1=mask[:],
                                    op=mybir.AluOpType.mult)
            enc = sb.tile([P, 1], F32)
            nc.vector.tensor_reduce(enc[:], glob[:],
                                    axis=mybir.AxisListType.X,
                                    op=mybir.AluOpType.max)
            idx_f = sb.tile([P, 1], F32)
            nc.vector.tensor_scalar(idx_f[0:SEG_PER_CORE, :], enc[0:SEG_PER_CORE, :],
                                    -1.0, float(L),
                                    op0=mybir.AluOpType.mult,
                                    op1=mybir.AluOpType.add)
            nc.vector.tensor_scalar_add(idx_f[SEG_PER_CORE:P, :],
                                        enc[SEG_PER_CORE:P, :], -1.0)
            idx_i = sb.tile([P, 1], I32)
            nc.vector.tensor_copy(idx_i[:], idx_f[:])
            rows = big.tile([P, H], F32)
            nc.gpsimd.indirect_dma_start(
                out=rows[:], out_offset=None, in_=x.ap(),
                in_offset=bass.IndirectOffsetOnAxis(ap=idx_i[:, 0:1], axis=0))
            nc.gpsimd.dma_start(out.ap()[:, 0:H], rows[0:SEG_PER_CORE, :])
            nc.sync.dma_start(out.ap()[:, H:2 * H], rows[SEG_PER_CORE:P, :])

    nc.compile()
    return nc


_NC = None


def _get_nc():
    global _NC
    if _NC is None:
        _NC = build_nc()
    return _NC


def make_in_maps(input, number_mask):
    x = np.ascontiguousarray(np.asarray(input), dtype=np.float32).reshape(L, H)
    nm = np.ascontiguousarray(np.asarray(number_mask))
    if nm.dtype != np.int64:
        nm = nm.astype(np.int64)
    idpairs = nm.reshape(L).view(np.int32).reshape(P, TOK_PER_PART, 2)
    c8lo = np.repeat(np.arange(8, dtype=np.float16), TOK_PER_PART)
    f = np.arange(TOK_PER_PART, dtype=np.float16)
    pcol = np.arange(P, dtype=np.float32)
    base = np.empty((2, SEG_PER_CORE, P), dtype=np.float32)
    base[0] = (P - 1 - pcol) * TOK_PER_PART
    base[1] = pcol * TOK_PER_PART
    in_maps = []
    for c in range(NCORES):
        c8hi = np.repeat(np.arange(8, dtype=np.float16) + c * 8, TOK_PER_PART)
        cpack = np.tile(np.concatenate([c8hi, c8lo, TOK_PER_PART - f, f + 1]),
                        (P, 1))
        in_maps.append({"x": x, "idpairs": idpairs, "cpack": cpack,
                        "base": base})
    return in_maps


def kernel(input, number_mask, n, concat, **_):
    assert int(n) == NSEG and int(concat) == 1
    nc = _get_nc()
    in_maps = make_in_maps(input, number_mask)
    res = bass_utils.run_bass_kernel_spmd(nc, in_maps, core_ids=list(range(NCORES)))
    return np.concatenate([res.results[c]["out"] for c in range(NCORES)], axis=0)



# revision 4
# speedup vs baseline: 2.1896x; 2.1896x over previous
"""Trainium2 Bass kernel for nn_AwareDecoder segment first/last gather.

Problem: input [16, 2048, 1024] f32, number_mask [16, 2048] int64 with ids in
[0, 512]. For each segment id i in [0, 512): find first/last row-major token
position with that id, gather those rows of the flattened input, concat ->
out [512, 2048] f32.

Strategy (8 NeuronCores, segment-sharded - no collectives):
  core c owns segments [64c, 64c+64). Each core:
    - DMAs the (tiny, 256KB) id array as int32 (lo,hi) pairs; chunk p =
      tokens [256p, 256p+256) lives on partition p,
    - bit-packs per-chunk presence of its 64 segments into 2 int32 words per
      chunk (eq-compare on the id high bits, variable left-shift by the id
      low bits, bitwise-OR tree over the 256 tokens) - ~8x less DVE work
      than a full eq/select/max sweep,
    - decodes first/last chunk per segment via bit-test + position encode +
      PE transpose + free-axis max-reduce,
    - gathers the candidate chunks' ids ON-CHIP with a one-hot PE matmul
      (no HBM round trip), then finds the exact within-chunk position with
      one compare + one fused tensor_tensor_reduce,
    - turns (chunk, pos) into global row indices and pulls its 64 first +
      64 last rows (4KB each) straight from HBM with one hardware indirect
      DMA (reads only 512KB of the 128MB input),
    - writes its [64, 2048] slice of the output.
Host concatenates the 8 slices.
"""
import numpy as np

import concourse.bass as bass
import concourse.tile as tile
from concourse import bacc, mybir
from concourse import bass_utils
from concourse.masks import make_identity

P = 128            # partitions / token chunks
L = 32768          # B*S tokens
H = 1024           # hidden
NSEG = 512         # segments
NCORES = 8
SEG_PER_CORE = NSEG // NCORES            # 64
TOK = L // P                             # 256 tokens per chunk
F32 = mybir.dt.float32
F16 = mybir.dt.float16
I32 = mybir.dt.int32

# cf16 layout (f16): [0:256] refine pos encode, [256] min-chunk encode
# (128-p), [257] max-chunk encode (p+1), [258:386] chunk iota 0..127
CFH_W = 386
# cf32 layout (f32): [0] side sign, [1] chunk decode const, [2] global decode
# const, [3] segment id, [4] word id 0, [5] word id 1
CFS_W = 6


def build_nc():
    nc = bacc.Bacc("TRN2", target_bir_lowering=False, debug=False)

    x = nc.dram_tensor("x", [L, H], F32, kind="ExternalInput")
    idpairs = nc.dram_tensor("idpairs", [P, TOK, 2], I32, kind="ExternalInput")
    cf16 = nc.dram_tensor("cf16", [P, CFH_W], F16, kind="ExternalInput")
    cf32 = nc.dram_tensor("cf32", [P, CFS_W], F32, kind="ExternalInput")
    ci32 = nc.dram_tensor("ci32", [P, SEG_PER_CORE], I32, kind="ExternalInput")
    out = nc.dram_tensor("out", [SEG_PER_CORE, 2 * H], F32, kind="ExternalOutput")

    with tile.TileContext(nc) as tc:
        with tc.tile_pool(name="sb", bufs=1) as sb, \
             tc.tile_pool(name="ps", bufs=1, space="PSUM") as ps:

            # ---- loads (parallel queues) ----
            idp = sb.tile([P, TOK, 2], I32)
            nc.sync.dma_start(idp[:], idpairs.ap())
            cfh = sb.tile([P, CFH_W], F16)
            nc.scalar.dma_start(cfh[:], cf16.ap())
            cfs = sb.tile([P, CFS_W], F32)
            nc.gpsimd.dma_start(cfs[:], cf32.ap())
            cis = sb.tile([P, SEG_PER_CORE], I32)
            nc.gpsimd.dma_start(cis[:], ci32.ap())
            ident = sb.tile([P, P], F16)
            make_identity(nc, ident[:])
            # f16 copy of ids for the PE gather (scalar engine, off the
            # DVE critical path)
            idsf = sb.tile([P, TOK], F16)
            nc.scalar.copy(idsf[:], idp[:, :, 0])

            # ---- main pass: bit-packed presence per (chunk, segment) ----
            lo5 = sb.tile([P, TOK], I32)
            nc.vector.tensor_scalar(lo5[:], idp[:, :, 0], 31, None,
                                    op0=mybir.AluOpType.bitwise_and)
            hi4 = sb.tile([P, TOK], I32)
            nc.vector.tensor_scalar(hi4[:], idp[:, :, 0], 5, None,
                                    op0=mybir.AluOpType.arith_shift_right)
            cand = sb.tile([P, 2, TOK], I32)
            eq0 = sb.tile([P, TOK], I32)
            nc.vector.tensor_scalar(eq0[:], hi4[:], cfs[:, 4:5], None,
                                    op0=mybir.AluOpType.is_equal)
            nc.vector.tensor_tensor(out=cand[:, 0], in0=eq0[:], in1=lo5[:],
                                    op=mybir.AluOpType.logical_shift_left)
            eq1 = sb.tile([P, TOK], I32)
            nc.vector.tensor_scalar(eq1[:], hi4[:], cfs[:, 5:6], None,
                                    op0=mybir.AluOpType.is_equal)
            nc.vector.tensor_tensor(out=cand[:, 1], in0=eq1[:], in1=lo5[:],
                                    op=mybir.AluOpType.logical_shift_left)
            # bitwise-OR tree over the token axis: 256 -> 1 per word
            lv = cand
            width = TOK
            while width > 1:
                half = width // 2
                nxt = sb.tile([P, 2, half], I32, tag=f"or{half}")
                nc.vector.tensor_tensor(out=nxt[:], in0=lv[:, :, 0:half],
                                        in1=lv[:, :, half:width],
                                        op=mybir.AluOpType.bitwise_or)
                lv = nxt
                width = half
            words = lv                                    # [P, 2, 1]

            # ---- decode: first/last chunk per segment ----
            bits_in = words[:, :, 0].unsqueeze(2).broadcast_to([P, 2, 32])
            cis_v = cis[:, 0:SEG_PER_CORE].rearrange("p (a b) -> p a b", a=2)
            andm = sb.tile([P, 2, 32], I32)
            nc.vector.tensor_tensor(out=andm[:], in0=bits_in, in1=cis_v,
                                    op=mybir.AluOpType.bitwise_and)
            pres = sb.tile([P, SEG_PER_CORE], F16)
            nc.vector.tensor_scalar(pres[:],
                                    andm[:].rearrange("p a b -> p (a b)"),
                                    0, None, op0=mybir.AluOpType.not_equal)
            enc = sb.tile([P, P], F16)
            nc.vector.tensor_tensor(
                out=enc[:, 0:SEG_PER_CORE], in0=pres[:],
                in1=cfh[:, 256:257].broadcast_to([P, SEG_PER_CORE]),
                op=mybir.AluOpType.mult)
            nc.vector.tensor_tensor(
                out=enc[:, SEG_PER_CORE:P], in0=pres[:],
                in1=cfh[:, 257:258].broadcast_to([P, SEG_PER_CORE]),
                op=mybir.AluOpType.mult)
            enc_t = ps.tile([P, P], F16)
            nc.tensor.transpose(out=enc_t[:], in_=enc[:], identity=ident[:])
            val = sb.tile([P, 1], F32)
            nc.vector.tensor_reduce(val[:], enc_t[:],
                                    axis=mybir.AxisListType.X,
                                    op=mybir.AluOpType.max)
            # candidate chunk per row: clamp(sgn*val + cstc, [0, 127])
            offf = sb.tile([P, 1], F32)
            nc.vector.tensor_tensor(out=offf[:], in0=val[:], in1=cfs[:, 0:1],
                                    op=mybir.AluOpType.mult)
            nc.vector.tensor_tensor(out=offf[:], in0=offf[:], in1=cfs[:, 1:2],
                                    op=mybir.AluOpType.add)
            offh = sb.tile([P, 1], F16)
            nc.vector.tensor_scalar(offh[:], offf[:], 127.0, 0.0,
                                    op0=mybir.AluOpType.min,
                                    op1=mybir.AluOpType.max)

            # ---- on-chip gather of candidate chunks' ids via PE ----
            onehot = sb.tile([P, P], F16)
            nc.vector.tensor_tensor(out=onehot[:], in0=cfh[:, 258:386],
                                    in1=offh[:].broadcast_to([P, P]),
                                    op=mybir.AluOpType.is_equal)
            onehot_t = ps.tile([P, P], F16)
            nc.tensor.transpose(out=onehot_t[:], in_=onehot[:],
                                identity=ident[:])
            onehot_s = sb.tile([P, P], F16)
            nc.scalar.copy(onehot_s[:], onehot_t[:])
            grows = ps.tile([P, TOK], F32)
            nc.tensor.matmul(grows[:], onehot_s[:], idsf[:],
                             start=True, stop=True)

            # ---- refine: exact within-chunk position ----
            eqr = sb.tile([P, TOK], F16)
            nc.vector.tensor_tensor(out=eqr[:], in0=grows[:],
                                    in1=cfs[:, 3:4].broadcast_to([P, TOK]),
                                    op=mybir.AluOpType.is_equal)
            encr = sb.tile([P, TOK], F16)
            nc.vector.tensor_tensor(out=encr[:], in0=eqr[:], in1=cfh[:, 0:TOK],
                                    op=mybir.AluOpType.mult)
            val2 = sb.tile([P, 1], F32)
            nc.vector.tensor_reduce(val2[:], encr[:],
                                    axis=mybir.AxisListType.X,
                                    op=mybir.AluOpType.max)

            # ---- global row index: clamp(sgn*(256*val + val2) + cstg) ----
            g = sb.tile([P, 1], F32)
            nc.vector.tensor_scalar(g[:], val[:], float(TOK), None,
                                    op0=mybir.AluOpType.mult)
            nc.vector.tensor_tensor(out=g[:], in0=g[:], in1=val2[:],
                                    op=mybir.AluOpType.add)
            nc.vector.tensor_tensor(out=g[:], in0=g[:], in1=cfs[:, 0:1],
                                    op=mybir.AluOpType.mult)
            nc.vector.tensor_tensor(out=g[:], in0=g[:], in1=cfs[:, 2:3],
                                    op=mybir.AluOpType.add)
            nc.vector.tensor_scalar(g[:], g[:], float(L - 1), 0.0,
                                    op0=mybir.AluOpType.min,
                                    op1=mybir.AluOpType.max)
            gi = sb.tile([P, 1], I32)
            nc.vector.tensor_copy(gi[:], g[:])

            # ---- gather rows, write out ----
            rows = sb.tile([P, H], F32)
            nc.gpsimd.indirect_dma_start(
                out=rows[:], out_offset=None, in_=x.ap(),
                in_offset=bass.IndirectOffsetOnAxis(ap=gi[:, 0:1], axis=0))
            nc.sync.dma_start(out.ap()[:, 0:H], rows[0:SEG_PER_CORE, :])
            nc.scalar.dma_start(out.ap()[:, H:2 * H], rows[SEG_PER_CORE:P, :])

    nc.compile()
    return nc


_NC = None


def _get_nc():
    global _NC
    if _NC is None:
        _NC = build_nc()
    return _NC


def make_in_maps(input, number_mask):
    x = np.ascontiguousarray(np.asarray(input), dtype=np.float32).reshape(L, H)
    nm = np.ascontiguousarray(np.asarray(number_mask))
    if nm.dtype != np.int64:
        nm = nm.astype(np.int64)
    idpairs = nm.reshape(L).view(np.int32).reshape(P, TOK, 2)

    r = np.arange(P)
    side_max = r >= SEG_PER_CORE                  # rows 0-63 min, 64-127 max
    t = np.arange(TOK, dtype=np.float16)
    cf16 = np.zeros((P, CFH_W), dtype=np.float16)
    cf16[:SEG_PER_CORE, 0:TOK] = TOK - t          # refine encode, min side
    cf16[SEG_PER_CORE:, 0:TOK] = t + 1            # refine encode, max side
    cf16[:, 256] = (P - r).astype(np.float16)     # min-chunk encode coeff
    cf16[:, 257] = (r + 1).astype(np.float16)     # max-chunk encode coeff
    cf16[:, 258:386] = np.arange(P, dtype=np.float16)[None, :]

    maskbits = np.tile((np.int32(1) << (np.arange(SEG_PER_CORE, dtype=np.int32)
                                        % 32)), (P, 1)).astype(np.int32)

    in_maps = []
    for c in range(NCORES):
        cf32 = np.zeros((P, CFS_W), dtype=np.float32)
        cf32[:, 0] = np.where(side_max, 1.0, -1.0)          # sgn
        cf32[:, 1] = np.where(side_max, -1.0, float(P))     # cstc
        cf32[:, 2] = np.where(side_max, -(TOK + 1.0), float(P * TOK + TOK))
        cf32[:, 3] = c * SEG_PER_CORE + (r % SEG_PER_CORE)  # segment id
        cf32[:, 4] = 2 * c                                  # word id 0
        cf32[:, 5] = 2 * c + 1                              # word id 1
        in_maps.append({"x": x, "idpairs": idpairs, "cf16": cf16,
                        "cf32": cf32, "ci32": maskbits})
    return in_maps


def kernel(input, number_mask, n, concat, **_):
    assert int(n) == NSEG and int(concat) == 1
    nc = _get_nc()
    in_maps = make_in_maps(input, number_mask)
    res = bass_utils.run_bass_kernel_spmd(nc, in_maps, core_ids=list(range(NCORES)))
    return np.concatenate([res.results[c]["out"] for c in range(NCORES)], axis=0)
